# revision 7
# baseline (speedup 1.0000x reference)
"""Trainium2 Bass kernel for nn_MultiHeadAttention (B=4, S=2048, D=512, H=8).

Sharding: tensor-parallel over heads — core c owns head c (Dh=64).
Each core computes q/k/v projections for its head slice (full x replicated),
attention for its head over all 4 batches, and the partial out-projection
O_c @ Wo[c]; the host sums the 8 partials (the TP all-reduce done at gather
time) and adds the biases that commute with that reduction (bo, bv@Wo).

Per-core pipeline (all matmul operands float32r = full-rate fp32):
  1. x[b] loads naturally; x^T chunks built with PE identity-transposes.
  2. Q^T, K^T [64, S] via W-stationary matmuls (+ bq/bk at PSUM evac).
  3. V^T via W-stationary (+ bv at evac... bv kept on host), PE-transposed
     into V_aug [128k, 65] tiles with a ones column (softmax denominators).
  4. S^T[k,q] = K^T.T @ Q^T (dh contraction); exp(S/8) on ACT, no max
     subtraction (|logits| < ~3 by construction).
  5. O^T_aug[65, q] += V_aug.T @ P^T — row 64 accumulates the denominators.
  6. Out-proj: lhsT = O^T_aug chunk, rhs = [Wo_c; 0] plus an e-column that
     lands the denominator on the token partition; DVE divides during evac.
"""
import numpy as np

import concourse.bass as bass
import concourse.mybir as mybir
import concourse.tile as tile
from concourse import bacc
from concourse.bass_utils import run_bass_kernel_spmd

B, S, D = 4, 2048, 512
H, DH = 8, 64
NCORES = 8
F32 = mybir.dt.float32
F32R = mybir.dt.float32r
AF = mybir.ActivationFunctionType

_NC_CACHE = {}


def xt_c_slice(xt, ci, blk):
    return xt[ci][:, bass.ts(blk, 512)]


def build_kernel():
    nc = bacc.Bacc("TRN2", target_bir_lowering=False, debug=False)

    x = nc.dram_tensor("x", [B, S, D], F32R, kind="ExternalInput")
    wq = nc.dram_tensor("wq", [D, DH], F32R, kind="ExternalInput")
    wk = nc.dram_tensor("wk", [D, DH], F32R, kind="ExternalInput")
    wv = nc.dram_tensor("wv", [D, DH], F32R, kind="ExternalInput")
    wo_aug = nc.dram_tensor("wo_aug", [DH + 2, D + 2], F32R, kind="ExternalInput")
    bq = nc.dram_tensor("bq", [DH, 1], F32, kind="ExternalInput")
    bk = nc.dram_tensor("bk", [DH, 1], F32, kind="ExternalInput")
    idin = nc.dram_tensor("idin", [128, 128], F32R, kind="ExternalInput")
    onesin = nc.dram_tensor("onesin", [128, 16, 2], F32R, kind="ExternalInput")
    out = nc.dram_tensor("out", [B * S, D], F32, kind="ExternalOutput")

    NKT = S // 128          # 16 k/token tiles per batch
    NQB = S // 512          # 4 512-blocks per batch
    NCH = D // 128          # 4 dm chunks

    with tile.TileContext(nc) as tc:
        with (
            tc.tile_pool(name="consts", bufs=1) as consts,
            tc.tile_pool(name="xnp", bufs=6) as xnp,
            tc.tile_pool(name="xtp", bufs=8) as xtp,
            tc.tile_pool(name="qkp", bufs=2) as qkp,
            tc.tile_pool(name="vtp", bufs=2) as vtp,
            tc.tile_pool(name="vp", bufs=2) as vp,
            tc.tile_pool(name="ptp", bufs=3) as ptp,
            tc.tile_pool(name="otp", bufs=2) as otp,
            tc.tile_pool(name="outp", bufs=4) as outp,
            tc.tile_pool(name="rcp", bufs=4) as rcp,
            tc.tile_pool(name="psA", bufs=2, space="PSUM") as psA,
            tc.tile_pool(name="psB", bufs=4, space="PSUM") as psB,
        ):
            # --- constants ---
            wq_sb = consts.tile([128, NCH, DH], F32R)
            wk_sb = consts.tile([128, NCH, DH], F32R)
            wv_sb = consts.tile([128, NCH, DH], F32R)
            wo_sb = consts.tile([DH + 2, D + 2], F32R)
            bq_sb = consts.tile([DH, 1], F32)
            bk_sb = consts.tile([DH, 1], F32)
            ident = consts.tile([128, 128], F32R)
            nc.sync.dma_start(out=wq_sb[:], in_=wq.rearrange("(c p) m -> p c m", p=128))
            nc.sync.dma_start(out=wk_sb[:], in_=wk.rearrange("(c p) m -> p c m", p=128))
            nc.sync.dma_start(out=wv_sb[:], in_=wv.rearrange("(c p) m -> p c m", p=128))
            nc.sync.dma_start(out=wo_sb[:], in_=wo_aug[:])
            nc.sync.dma_start(out=bq_sb[:], in_=bq[:])
            nc.sync.dma_start(out=bk_sb[:], in_=bk[:])
            nc.sync.dma_start(out=ident[:], in_=idin[:])

            for b in range(B):
                # --- load x naturally; build x^T chunks via PE transpose ---
                xt = []
                for ci in range(NCH):
                    xt_c = xtp.tile([128, S], F32R, tag="xt", name=f"xt_{b}_{ci}")
                    xt.append(xt_c)
                for blk in range(NQB):
                    xn4 = []
                    for j in range(4):
                        xn_t = xnp.tile([128, D], F32R, tag="xn", name=f"xn_{b}_{blk}_{j}")
                        nc.sync.dma_start(
                            out=xn_t[:], in_=x[b, bass.ds(blk * 512 + j * 128, 128), :]
                        )
                        xn4.append(xn_t)
                    for ci in range(NCH):
                        pxt = psA.tile([128, 512], F32R, tag="psA")
                        for j in range(4):
                            nc.tensor.transpose(
                                pxt[:, bass.ts(j, 128)],
                                xn4[j][:, bass.ts(ci, 128)],
                                ident[:],
                            )
                        nc.vector.tensor_copy(xt_c_slice(xt, ci, blk), pxt[:])

                # --- Q^T, K^T projections (W-stationary) ---
                qt_b = qkp.tile([DH, S], F32R, tag="qt")
                kt_b = qkp.tile([DH, S], F32R, tag="kt")
                for blk in range(NQB):
                    sl = bass.ts(blk, 512)
                    pq = psB.tile([DH, 512], F32, tag="psB")
                    pk = psB.tile([DH, 512], F32, tag="psB")
                    for ci in range(NCH):
                        nc.tensor.matmul(
                            pq[:], wq_sb[:, ci, :], xt[ci][:, sl],
                            start=(ci == 0), stop=(ci == NCH - 1),
                        )
                    for ci in range(NCH):
                        nc.tensor.matmul(
                            pk[:], wk_sb[:, ci, :], xt[ci][:, sl],
                            start=(ci == 0), stop=(ci == NCH - 1),
                        )
                    nc.scalar.activation(qt_b[:, sl], pq[:], AF.Identity, bias=bq_sb[:])
                    nc.scalar.activation(kt_b[:, sl], pk[:], AF.Identity, bias=bk_sb[:])

                # --- V^T projection, then PE-transpose into V_aug ---
                vt_b = vtp.tile([DH, S], F32R, tag="vt")
                for blk in range(NQB):
                    sl = bass.ts(blk, 512)
                    pvt = psB.tile([DH, 512], F32, tag="psB")
                    for ci in range(NCH):
                        nc.tensor.matmul(
                            pvt[:], wv_sb[:, ci, :], xt[ci][:, sl],
                            start=(ci == 0), stop=(ci == NCH - 1),
                        )
                    nc.scalar.copy(vt_b[:, sl], pvt[:])
                v_b = vp.tile([128, NKT, DH + 2], F32R, tag="v")
                nc.sync.dma_start(out=v_b[:, :, DH:DH + 2], in_=onesin[:])
                for half in range(2):
                    pvtr = psA.tile([128, 512], F32R, tag="psA")
                    for j in range(8):
                        nc.tensor.transpose(
                            pvtr[:, bass.ts(j, 64)],
                            vt_b[:, bass.ts(half * 8 + j, 128)],
                            ident[0:DH, 0:DH],
                        )
                    nc.vector.tensor_copy(
                        v_b[:, bass.ds(half * 8, 8), 0:DH],
                        pvtr[:].rearrange("p (k m) -> p k m", m=64),
                    )

                # --- attention: S^T -> exp -> O^T_aug ---
                ot_b = otp.tile([DH + 2, S], F32R, tag="ot")
                for qh in range(2):  # 1024-wide q halves
                    po = [
                        psB.tile([DH + 2, 512], F32, tag="psB", name=f"po{qh}_{j}")
                        for j in range(2)
                    ]
                    for kt_i in range(NKT):
                        pst = psA.tile([128, 1024], F32, tag="psA")
                        for j in range(2):
                            nc.tensor.matmul(
                                pst[:, bass.ts(j, 512)],
                                kt_b[:, bass.ts(kt_i, 128)],
                                qt_b[:, bass.ds(qh * 1024 + j * 512, 512)],
                                start=True, stop=True,
                            )
                        ptt = ptp.tile([128, 1024], F32R, tag="pt")
                        nc.scalar.activation(ptt[:], pst[:], AF.Exp, scale=0.125)
                        for j in range(2):
                            nc.tensor.matmul(
                                po[j][:], v_b[:, kt_i, :], ptt[:, bass.ts(j, 512)],
                                start=(kt_i == 0), stop=(kt_i == NKT - 1),
                            )
                    for j in range(2):
                        nc.scalar.copy(
                            ot_b[:, bass.ds(qh * 1024 + j * 512, 512)], po[j][:]
                        )

                # --- out-projection + normalize ---
                for tt in range(NKT):
                    pop = psB.tile([128, 512], F32, tag="psB")
                    pos = psB.tile([128, 2], F32, tag="psB")
                    otc = ot_b[:, bass.ts(tt, 128)]
                    nc.tensor.matmul(pop[:], otc, wo_sb[:, 0:D], start=True, stop=True)
                    nc.tensor.matmul(pos[:], otc, wo_sb[:, D:D + 2], start=True, stop=True)
                    rc = rcp.tile([128, 1], F32, tag="rc")
                    nc.vector.reciprocal(rc[:], pos[:, 0:1])
                    so = outp.tile([128, 512], F32, tag="so")
                    nc.vector.tensor_scalar_mul(so[:], pop[:], rc[:])
                    nc.sync.dma_start(
                        out=out[bass.ds(b * S + tt * 128, 128), :], in_=so[:]
                    )

    nc.compile()
    return nc


def kernel(x, Wq, bq, Wk, bk, Wv, bv, Wo, bo):
    x = np.ascontiguousarray(np.asarray(x, dtype=np.float32))
    Wq = np.asarray(Wq, dtype=np.float32)
    Wk = np.asarray(Wk, dtype=np.float32)
    Wv = np.asarray(Wv, dtype=np.float32)
    Wo = np.asarray(Wo, dtype=np.float32)
    bq = np.asarray(bq, dtype=np.float32)
    bk = np.asarray(bk, dtype=np.float32)
    bv = np.asarray(bv, dtype=np.float32)
    bo = np.asarray(bo, dtype=np.float32)

    if "nc" not in _NC_CACHE:
        _NC_CACHE["nc"] = build_kernel()
    nc = _NC_CACHE["nc"]

    eye = np.eye(128, dtype=np.float32)
    ones = np.zeros((128, 16, 2), dtype=np.float32); ones[:, :, 0] = 1.0
    in_maps = []
    for c in range(NCORES):
        hs = slice(c * DH, (c + 1) * DH)
        wo_aug = np.zeros((DH + 2, D + 2), dtype=np.float32)
        wo_aug[0:DH, 0:D] = Wo[hs, :]
        wo_aug[DH, D] = 1.0
        in_maps.append({
            "x": x,
            "wq": np.ascontiguousarray(Wq[:, hs]),
            "wk": np.ascontiguousarray(Wk[:, hs]),
            "wv": np.ascontiguousarray(Wv[:, hs]),
            "wo_aug": wo_aug,
            "bq": np.ascontiguousarray(bq[hs].reshape(DH, 1)),
            "bk": np.ascontiguousarray(bk[hs].reshape(DH, 1)),
            "idin": eye,
            "onesin": ones,
        })

    res = run_bass_kernel_spmd(nc, in_maps, list(range(NCORES)))

    acc = np.zeros((B * S, D), dtype=np.float32)
    for c in range(NCORES):
        acc += res.results[c]["out"]
    # biases that commute with the head-reduction, applied at gather time
    acc += bo[None, :] + (bv @ Wo)[None, :]
    return acc.reshape(B, S, D)
